# revision 10
# baseline (speedup 1.0000x reference)
"""Trainium2 Bass kernel for nn_MoEGate_6150393168540 (moe_routing).

Computes, for x [B=65536, D=1024], gate/expert weights [E=8, D] and biases [E]:
    gate = softmax(x @ gate_w.T + gate_b)            # [B, 8]
    keep top-k (k=2) gate values, zero the rest (no renormalization)
    expert = x @ expert_w.T + expert_b               # [B, 8]
    out = sum(gate_masked * expert, axis=1)          # [B, 1]

Strategy (8 NeuronCores, data-parallel over the batch):
  - Each core gets 8192 rows of x; weights are replicated.
  - The D-contraction needs x transposed (D on partitions). PE transposes x in
    fp32 ([128,128] blocks -> PSUM, bit-exact, 2 cyc/row).
  - Scores must order-match a pure-fp32 reference (top-2 selection), so the
    matmul uses an exact fp16 Dekker split: hi = bf16-trunc(xT) (ACT bit-slice
    copy from PSUM), lo = fp16(xT - hi) (DVE). Weights are split host-side the
    same way. hi*w_hi, hi*w_lo, lo*w_hi are exact products accumulated in fp32
    PSUM => scores accurate to ~1e-7, at 16-bit matmul speed.
  - mm_hi: lhsT=[w_hi|w_lo] [128,32] -> psum rows 0:32; mm_lo accumulates.
  - A tiny "fold" matmul (lhsT = scores-pair fp16, rhs = [I16;I16]) both
    transposes scores back to batch-major and sums hi+lo partial rows.
  - Postprocess per 2 blocks (slice of the 8-block PSUM bank): +bias, exp on
    ACT, top-2 threshold via min/max tournament, mask, weighted sum, divide.
    A final DVE 32x32 transpose per 8-block group makes the output DMA
    contiguous.

Perf notes (vs the 181us first version):
  - x is DMAed in D-quarters with one tile per quarter so the first transpose
    only waits ~512KB, not the full 2MB block (head -8us).
  - prune_waits() removes semaphore waits implied by engine issue order or by
    the transitive closure of a producer's own waits; walrus only supports one
    wait per instruction, so fewer waits means fewer NOPs and less sem-eval
    latency per PE/ACT/DVE instruction.
  - postprocess runs per 2 blocks so the serial tail after the last fold is
    ~1.5us instead of ~5.8us.
"""

import sys

sys.path.insert(0, "/opt/trn_rl_repo")

from contextlib import ExitStack

import numpy as np

import concourse.bass as bass
import concourse.mybir as mybir
import concourse.tile as tile

F32 = mybir.dt.float32
F16 = mybir.dt.float16
BF16 = mybir.dt.bfloat16
U16 = mybir.dt.uint16
ALU = mybir.AluOpType
AXX = mybir.AxisListType.X
EXP = mybir.ActivationFunctionType.Exp

B, D, E = 65536, 1024, 8
N_CORES = 8
B_LOC = B // N_CORES  # 8192
BLK = 512  # rows per block
DC = D // 128  # 8 contraction chunks
PENDING = 6  # matmul stages queued behind the transpose front


def _join(dst: dict, src: dict):
    for k, v in src.items():
        if dst.get(k, -1) < v:
            dst[k] = v


def prune_waits(nc) -> int:
    """Remove semaphore waits that are implied transitively.

    Model: sems are monotone counters (sem-ge-imm waits, sem-inc updates).
    - Engine sequencers evaluate waits in program order, so instruction n+1
      inherits every guarantee instruction n's waits established (G).
    - A satisfied wait (s>=v) implies its producer COMPLETED, which implies
      the producer's own guarantees plus its updates plus (same engine /
      same DMA queue) all earlier completions (C).
    Only sems whose updaters all sit on one engine stream are used as
    evidence (per-queue DMA completion order == trigger order); waits on
    cross-core or mixed-engine sems are never pruned away implicitly and
    never pruned.
    """
    n_pruned = 0
    for f in nc.m.functions:
        for bb in f.blocks:
            insts = bb.instructions
            # per-sem updater order (program order of the updating engine)
            sem_updaters = {}  # sem_id -> [(inst_idx, cum_value)]
            sem_engines = {}
            for i, inst in enumerate(insts):
                si = inst.sync_info
                if not si or not si.on_update:
                    continue
                for u in si.on_update:
                    if u.sync_type != "semaphore" or u.update_mode != "sem-inc":
                        sem_engines[u.id] = "mixed"
                        continue
                    lst = sem_updaters.setdefault(u.id, [])
                    cum = (lst[-1][1] if lst else 0) + (u.update_value or 1)
                    lst.append((i, cum))
                    eng = sem_engines.setdefault(u.id, inst.engine)
                    if eng != inst.engine:
                        sem_engines[u.id] = "mixed"
            orderable = {
                s for s, e in sem_engines.items() if e != "mixed"
            }

            def producer(sem_id, value):
                if sem_id not in orderable:
                    return None
                for i, cum in sem_updaters.get(sem_id, []):
                    if cum >= value:
                        return i
                return None

            # async updaters: sem fires at DMA completion, which races with
            # later instructions on the issuing engine (only same-queue DMA
            # completions are ordered)
            def is_async(inst):
                # DMA triggers complete asynchronously; GpSimd (Pool) is only
                # per-Q7 FIFO, so treat its completions as unordered too.
                tn = type(inst).__name__
                return (
                    "DMA" in tn
                    or "Dma" in tn
                    or inst.engine == mybir.EngineType.Pool
                )

            # own_cum[i] = {sem: cum value} this instruction's updates reach
            own_cum = [dict() for _ in insts]
            queue_pred = [dict() for _ in insts]  # sem -> prev updater idx
            for s, lst in sem_updaters.items():
                if s not in orderable:
                    continue
                prev = None
                for i, cum in lst:
                    own_cum[i][s] = cum
                    if prev is not None:
                        queue_pred[i][s] = prev
                    prev = i

            # G[i]: guarantees when inst i is dispatched (after its waits).
            # C_eng[i]: what the next instruction on the same engine may
            #   assume about i (its dispatch guarantees; plus its own sync
            #   updates only if it is not an async DMA trigger).
            # C_full[i]: what a waiter observing i's sem update may assume
            #   (i completed: dispatch guarantees + same-queue predecessor
            #   completions + own updates).
            G = [None] * len(insts)
            C_eng = [None] * len(insts)
            C_full = [None] * len(insts)
            last_on_engine = {}
            for i, inst in enumerate(insts):
                si = inst.sync_info
                eng = inst.engine
                pred = last_on_engine.get(eng)
                g: dict = dict(G[pred]) if pred is not None else {}
                if si and si.on_wait:
                    waits = [
                        w
                        for w in si.on_wait
                        if w.sync_type == "semaphore"
                        and w.wait_mode == "sem-ge-imm"
                    ]
                    other = [
                        w
                        for w in si.on_wait
                        if not (
                            w.sync_type == "semaphore"
                            and w.wait_mode == "sem-ge-imm"
                        )
                    ]
                    # strongest (latest-producer) waits first
                    def lateness(w):
                        p = producer(w.id, w.wait_value)
                        return -1 if p is None else p

                    waits.sort(key=lateness, reverse=True)
                    kept = []
                    for w in waits:
                        # only monotone single-engine sems may be pruned or
                        # serve as evidence: barrier/event sems can be reset
                        # and reused, so nothing about them persists
                        if w.id not in orderable:
                            kept.append(w)
                            continue
                        if g.get(w.id, -1) >= w.wait_value:
                            n_pruned += 1
                            continue
                        kept.append(w)
                        g[w.id] = w.wait_value
                        p = producer(w.id, w.wait_value)
                        if p is not None and C_full[p] is not None:
                            _join(g, C_full[p])
                    if len(kept) != len(waits):
                        inst.sync_info = mybir.SyncInfo(
                            on_wait=other + kept,
                            on_update=list(si.on_update or []),
                        )
                G[i] = g
                async_i = is_async(inst)
                ce = dict(g)
                if pred is not None:
                    _join(ce, C_eng[pred])
                cf = dict(ce)
                if async_i:
                    cf = dict(g)
                    for s, qp in queue_pred[i].items():
                        if C_full[qp] is not None:
                            _join(cf, C_full[qp])
                for s, cum in own_cum[i].items():
                    if cf.get(s, -1) < cum:
                        cf[s] = cum
                    if not async_i and ce.get(s, -1) < cum:
                        ce[s] = cum
                C_eng[i] = ce
                C_full[i] = cf
                last_on_engine[eng] = i
    return n_pruned


def split_waits(nc, max_waits: int = 1) -> int:
    """walrus allows only one semaphore wait per instruction; hoist the
    rest into preceding single-wait NOPs on the same engine (engine streams
    execute in order, so earlier waits on the same engine are equivalent)."""
    n_split = 0
    for f in nc.m.functions:
        for bb in f.blocks:
            new = []
            for inst in bb.instructions:
                si = inst.sync_info
                if si is not None and si.on_wait and len(si.on_wait) > max_waits:
                    waits = list(si.on_wait)
                    for w in waits[:-max_waits]:
                        n_split += 1
                        nop = mybir.InstNoOp(name=f"{inst.name}-ws{n_split}")
                        nop.engine = inst.engine
                        nop.sync_info = mybir.SyncInfo(on_wait=[w], on_update=[])
                        new.append(nop)
                    inst.sync_info = mybir.SyncInfo(
                        on_wait=waits[-max_waits:], on_update=list(si.on_update or [])
                    )
                new.append(inst)
            bb.instructions = new
    return n_split


def build_module(b_loc: int = B_LOC, split: bool = True):
    assert b_loc % 4096 == 0
    nc = bass.Bass()
    x = nc.dram_tensor("x", [b_loc, D], F32, kind="ExternalInput")
    whl = nc.dram_tensor("whl", [DC, 128, 32], F16, kind="ExternalInput")
    brow = nc.dram_tensor("brow", [512], F32, kind="ExternalInput")
    fold = nc.dram_tensor("fold", [32, 16], F16, kind="ExternalInput")
    ident = nc.dram_tensor("ident", [128, 128], F32, kind="ExternalInput")
    y = nc.dram_tensor("y", [b_loc], F32, kind="ExternalOutput")

    n_blk = b_loc // BLK
    tt = nc.vector.tensor_tensor

    with tile.TileContext(nc) as tc, ExitStack() as ctx:
        consts = ctx.enter_context(tc.tile_pool(name="consts", bufs=1))
        xpool = ctx.enter_context(tc.tile_pool(name="xpool", bufs=3))
        xh_pool = ctx.enter_context(tc.tile_pool(name="xh", bufs=8))
        xl_pool = ctx.enter_context(tc.tile_pool(name="xl", bufs=8))
        z32_pool = ctx.enter_context(tc.tile_pool(name="z32", bufs=3))
        pp = ctx.enter_context(tc.tile_pool(name="pp", bufs=2))
        ov_pool = ctx.enter_context(tc.tile_pool(name="ov", bufs=2))
        xt_pool = ctx.enter_context(tc.tile_pool(name="xtps", bufs=5, space="PSUM"))
        z_pool = ctx.enter_context(tc.tile_pool(name="zps", bufs=2, space="PSUM"))
        zt_pool = ctx.enter_context(tc.tile_pool(name="ztps", bufs=1, space="PSUM"))

        ident_sb = consts.tile([128, 128], F32)
        whl_sb = consts.tile([128, DC, 32], F16)
        fold_sb = consts.tile([32, 16], F16)
        bias_sb = consts.tile([128, 512], F32)

        def pp_part(zt_ps, pair_i, outv):
            """Elementwise/reduce postprocess of 2 blocks = cols
            [128*pair_i : 128*pair_i+128) of zt_ps -> outv[:, 8p:8p+8]."""
            c0 = 128 * pair_i
            zsl = zt_ps[:, c0 : c0 + 128].rearrange("p (g e) -> p g e", e=16)
            bsl = bias_sb[:, c0 : c0 + 128].rearrange("p (g e) -> p g e", e=16)
            zb = pp.tile([128, 8, 16], F32)
            nc.vector.tensor_add(zb, zsl, bsl)
            g8 = zb[:, :, 0:8]
            y8 = zb[:, :, 8:16]
            p8 = pp.tile([128, 8, 8], F32)
            nc.scalar.activation(p8, g8, EXP)
            den = pp.tile([128, 8], F32)
            nc.vector.tensor_reduce(den, p8, axis=AXX, op=ALU.add)
            # top-2 threshold: tournament keeping (max, 2nd max) per segment
            h1 = pp.tile([128, 8, 4], F32)
            l1 = pp.tile([128, 8, 4], F32)
            tt(h1, p8[:, :, 0:4], p8[:, :, 4:8], op=ALU.max)
            tt(l1, p8[:, :, 0:4], p8[:, :, 4:8], op=ALU.min)
            h2 = pp.tile([128, 8, 2], F32)
            v2 = pp.tile([128, 8, 2], F32)
            u2 = pp.tile([128, 8, 2], F32)
            m2q = pp.tile([128, 8, 2], F32)
            tt(h2, h1[:, :, 0:2], h1[:, :, 2:4], op=ALU.max)
            tt(v2, h1[:, :, 0:2], h1[:, :, 2:4], op=ALU.min)
            tt(u2, l1[:, :, 0:2], l1[:, :, 2:4], op=ALU.max)
            tt(m2q, u2, v2, op=ALU.max)
            v3 = pp.tile([128, 8, 1], F32)
            u3 = pp.tile([128, 8, 1], F32)
            m2f = pp.tile([128, 8, 1], F32)
            tt(v3, h2[:, :, 0:1], h2[:, :, 1:2], op=ALU.min)
            tt(u3, m2q[:, :, 0:1], m2q[:, :, 1:2], op=ALU.max)
            tt(m2f, u3, v3, op=ALU.max)
            # mask & weighted sum
            msk = pp.tile([128, 8, 8], F32)
            tt(msk, p8, m2f.to_broadcast([128, 8, 8]), op=ALU.is_ge)
            pm = pp.tile([128, 8, 8], F32)
            tt(pm, msk, p8, op=ALU.mult)
            prod = pp.tile([128, 8, 8], F32)
            tt(prod, pm, y8, op=ALU.mult)
            num = pp.tile([128, 8], F32)
            nc.vector.tensor_reduce(num, prod, axis=AXX, op=ALU.add)
            rden = pp.tile([128, 8], F32)
            nc.vector.reciprocal(rden, den)
            tt(outv[:, 8 * pair_i : 8 * pair_i + 8], num, rden, op=ALU.mult)

        def pp_finish(outv, b0):
            # 32x32 block transpose so each partition holds one contiguous run
            tv = pp.tile([128, 32], F32)
            nc.vector.transpose(tv, outv)
            yf = y.ap()
            for a in range(4):
                # dest[i, k] = y[b0 + 32a + 128 i + k], i,k in 0..32
                dest = bass.AP(yf.tensor, b0 + 32 * a, [[128, 32], [1, 32]])
                nc.sync.dma_start(out=dest, in_=tv[32 * a : 32 * a + 32, :])

        state = {"zt_ps": None, "outv": None}

        def emit_stage(blk, c, z_ps, xh, xl):
            # hi stream (bf16 view) and lo stream (fp16), both against the
            # full [w_hi|w_lo] fp16 pair: together they accumulate the
            # complete product (hi+lo)*(w_hi+w_lo) into rows 0:32.
            nc.tensor.matmul(
                z_ps, whl_sb[:, c, :], xh.bitcast(BF16),
                start=(c == 0), stop=False,
            )
            nc.tensor.matmul(
                z_ps, whl_sb[:, c, :], xl, start=False, stop=(c == DC - 1)
            )
            if c == DC - 1:
                # block tail: scores as an exact fp16 (hi, lo) pair so the
                # fold matmuls get single-pass fp16 weight loads
                zz = z32_pool.tile([32, 1024], F16)
                nc.scalar.copy(zz[:, 0:512], z_ps)
                nc.vector.tensor_sub(zz[:, 512:1024], z_ps, zz[:, 0:512])
                bank_i = blk % 8
                if bank_i == 0:
                    state["zt_ps"] = zt_pool.tile(
                        [128, 512], F32, name="zt_ps", tag="zt_ps"
                    )
                    state["outv"] = ov_pool.tile(
                        [128, 32], F32, name="outv", tag="outv"
                    )
                zt_ps = state["zt_ps"]
                for j in range(4):
                    col = (bank_i * 4 + j) * 16
                    nc.tensor.matmul(
                        zt_ps[:, col : col + 16],
                        zz[:, 128 * j : 128 * j + 128],
                        fold_sb,
                        start=True,
                        stop=False,
                    )
                    nc.tensor.matmul(
                        zt_ps[:, col : col + 16],
                        zz[:, 512 + 128 * j : 512 + 128 * j + 128],
                        fold_sb,
                        start=False,
                        stop=True,
                    )
                if bank_i % 2 == 1:
                    psave = tc.cur_priority
                    tc.cur_priority = psave + 100000
                    pp_part(zt_ps, bank_i // 2, state["outv"])
                    if bank_i == 7:
                        pp_finish(state["outv"], (blk // 8) * 4096)
                    tc.cur_priority = psave

        pending = []
        for blk in range(n_blk):
            r0 = blk * BLK
            xin = x.ap()[r0 : r0 + BLK, :].rearrange("(j p) d -> p j d", p=128)
            # one tile per D-half: a half is exactly one 4-chunk transpose
            # group, so a group's data becomes ready atomically and the ring
            # kick count stays low (each dma_start costs ~0.7us of sequencer
            # time and its completion sem lags the 16-way-split transfer)
            xq = []
            for h in range(2):
                xq.append(xpool.tile([128, 4, 512], F32, name=f"xh{h}"))
            if blk == 0:
                # block 0: tiny chunk-0 piece first so the first transpose
                # can start as early as possible; consts ride the Activation
                # ring in parallel; bias the gpsimd one
                nc.sync.dma_start(out=ident_sb, in_=ident.ap())
                nc.sync.dma_start(out=xq[0][:, :, 0:128], in_=xin[:, :, 0:128])
                nc.scalar.dma_start(out=whl_sb, in_=whl.ap().transpose([1, 0, 2]))
                nc.sync.dma_start(
                    out=xq[0][:, :, 128:512], in_=xin[:, :, 128:512]
                )
                nc.scalar.dma_start(out=fold_sb, in_=fold.ap())
                nc.gpsimd.dma_start(
                    out=bias_sb,
                    in_=brow.ap().unsqueeze(0).to_broadcast([128, 512]),
                )
                nc.sync.dma_start(out=xq[1], in_=xin[:, :, 512:1024])
            else:
                for h in range(2):
                    nc.sync.dma_start(
                        out=xq[h], in_=xin[:, :, 512 * h : 512 * h + 512]
                    )
            z_ps = z_pool.tile([32, 512], F32)
            # batch 4 chunks of transposes, then that many matmul stages:
            # transpose-mode <-> matmul-mode switches cost ~100ns each on
            # the PE, so alternating per chunk wastes ~25us/core
            for g in range(DC // 4):
                for c in range(4 * g, 4 * g + 4):
                    xt_ps = xt_pool.tile([128, 512], F32)
                    src = xq[c // 4]
                    d0 = (c % 4) * 128
                    for j in range(4):
                        nc.tensor.transpose(
                            xt_ps[:, 128 * j : 128 * j + 128],
                            src[:, j, d0 : d0 + 128],
                            ident_sb,
                        )
                    # hi = bf16 truncation of xT: u16 bit-slice copy on ACT
                    # (psum -> sbuf); lo = fp16(xT - hi) on DVE.
                    xt_hi_view = (
                        xt_ps.bitcast(U16)
                        .rearrange("p (k two) -> p k two", two=2)[:, :, 1]
                    )
                    xh = xh_pool.tile([128, 512], U16)
                    nc.scalar.copy(xh, xt_hi_view)
                    xl = xl_pool.tile([128, 512], F16)
                    nc.vector.tensor_sub(xl, xt_ps, xh.bitcast(BF16))
                    pending.append((blk, c, z_ps, xh, xl))
                while len(pending) > 4:
                    emit_stage(*pending.pop(0))
        for args in pending:
            emit_stage(*args)

    if split:
        import os

        n_pruned = 0
        if not os.environ.get("MOE_NO_PRUNE"):
            n_pruned = prune_waits(nc)
        n_split = split_waits(nc)
        if os.environ.get("MOE_DEBUG"):
            print(f"prune_waits: {n_pruned} pruned; split_waits: {n_split} NOPs")
    return nc


def host_inputs(gate_w, gate_b, expert_w, expert_b):
    """Host-side prep of the small replicated tensors."""
    W = np.concatenate([gate_w, expert_w], axis=0).astype(np.float32)  # [16, D]
    WT = W.T  # [D, 16]
    w_hi = WT.astype(np.float16)
    w_lo = (WT - w_hi.astype(np.float32)).astype(np.float16)
    whl = np.empty((DC, 128, 32), dtype=np.float16)
    for c in range(DC):
        whl[c, :, 0:16] = w_hi[128 * c : 128 * (c + 1), :]
        whl[c, :, 16:32] = w_lo[128 * c : 128 * (c + 1), :]
    bcat = np.concatenate([gate_b, expert_b]).astype(np.float32)  # [16]
    brow = np.tile(bcat, 32)  # [512]
    fold = np.concatenate([np.eye(16), np.eye(16)], axis=0).astype(np.float16)
    ident = np.eye(128, dtype=np.float32)
    return {"whl": whl, "brow": brow, "fold": fold, "ident": ident}


_NC_CACHE = {}


def kernel(x, gate_w, gate_b, expert_w, expert_b, k):
    assert int(k) == 2
    x = np.ascontiguousarray(np.asarray(x, dtype=np.float32))
    assert x.shape == (B, D)

    from concourse.bass_utils import run_bass_kernel_spmd

    if B_LOC not in _NC_CACHE:
        _NC_CACHE[B_LOC] = build_module(B_LOC)
    nc = _NC_CACHE[B_LOC]

    common = host_inputs(
        np.asarray(gate_w, np.float32),
        np.asarray(gate_b, np.float32),
        np.asarray(expert_w, np.float32),
        np.asarray(expert_b, np.float32),
    )
    in_maps = [
        {**common, "x": x[i * B_LOC : (i + 1) * B_LOC]} for i in range(N_CORES)
    ]
    import os

    trace = bool(os.environ.get("MOE_TRACE"))
    if trace:
        _ensure_ntff_hook()
    res = run_bass_kernel_spmd(
        nc, in_maps, core_ids=list(range(N_CORES)), trace=trace
    )
    global LAST_RESULT
    LAST_RESULT = res
    out = np.concatenate([r["y"] for r in res.results])
    return out.reshape(B, 1).astype(np.float32)


LAST_RESULT = None


def _ensure_ntff_hook():
    """Register the axon NTFF profile hook if the antenv shim is missing
    (lets run_bass_kernel_spmd(trace=True) capture HW timing)."""
    try:
        import antenv.axon_hooks  # noqa: F401

        return
    except ImportError:
        pass
    try:
        import types

        import antenv
        from trn_agent_boot.trn_boot import _ntff_profile_via_ctypes

        mod = types.ModuleType("antenv.axon_hooks")
        _h = [None]
        mod.set_axon_ntff_profile_hook = lambda h: _h.__setitem__(0, h)
        mod.get_axon_ntff_profile_hook = lambda: _h[0]
        sys.modules["antenv.axon_hooks"] = mod
        antenv.axon_hooks = mod
        mod.set_axon_ntff_profile_hook(
            _ntff_profile_via_ctypes("/opt/axon/libaxon_pjrt.so")
        )
    except Exception as e:  # profiling is best-effort
        print(f"ntff hook setup failed: {e}")


if __name__ == "__main__":
    rng = np.random.default_rng(0)
    s = 1.0 / np.sqrt(D)
    inputs = {
        "x": rng.standard_normal((B, D), dtype=np.float32),
        "gate_w": rng.uniform(-s, s, (E, D)).astype(np.float32),
        "gate_b": rng.uniform(-s, s, E).astype(np.float32),
        "expert_w": rng.uniform(-s, s, (E, D)).astype(np.float32),
        "expert_b": rng.uniform(-s, s, E).astype(np.float32),
        "k": 2,
    }
    got = kernel(**inputs)
    print("kernel output:", got.shape, got.dtype, got[:4, 0])


# revision 11
# speedup vs baseline: 1.0160x; 1.0160x over previous
"""Trainium2 Bass kernel for nn_MoEGate_6150393168540 (moe_routing).

Computes, for x [B=65536, D=1024], gate/expert weights [E=8, D] and biases [E]:
    gate = softmax(x @ gate_w.T + gate_b)            # [B, 8]
    keep top-k (k=2) gate values, zero the rest (no renormalization)
    expert = x @ expert_w.T + expert_b               # [B, 8]
    out = sum(gate_masked * expert, axis=1)          # [B, 1]

Strategy (8 NeuronCores, data-parallel over the batch):
  - Each core gets 8192 rows of x; weights are replicated.
  - The D-contraction needs x transposed (D on partitions). PE transposes x in
    fp32 ([128,128] blocks -> PSUM, bit-exact, 2 cyc/row).
  - Scores must order-match a pure-fp32 reference (top-2 selection), so the
    matmul uses an exact fp16 Dekker split: hi = bf16-trunc(xT) (ACT bit-slice
    copy from PSUM), lo = fp16(xT - hi) (DVE). Weights are split host-side the
    same way. hi*w_hi, hi*w_lo, lo*w_hi are exact products accumulated in fp32
    PSUM => scores accurate to ~1e-7, at 16-bit matmul speed.
  - mm_hi: lhsT=[w_hi|w_lo] [128,32] -> psum rows 0:32; mm_lo accumulates.
  - A tiny "fold" matmul (lhsT = scores-pair fp16, rhs = [I16;I16]) both
    transposes scores back to batch-major and sums hi+lo partial rows.
  - Postprocess per 2 blocks (slice of the 8-block PSUM bank): +bias, exp on
    ACT, top-2 threshold via min/max tournament, mask, weighted sum, divide.
    A final DVE 32x32 transpose per 8-block group makes the output DMA
    contiguous.

Perf notes (vs the 181us first version):
  - x is DMAed in D-quarters with one tile per quarter so the first transpose
    only waits ~512KB, not the full 2MB block (head -8us).
  - prune_waits() removes semaphore waits implied by engine issue order or by
    the transitive closure of a producer's own waits; walrus only supports one
    wait per instruction, so fewer waits means fewer NOPs and less sem-eval
    latency per PE/ACT/DVE instruction.
  - postprocess runs per 2 blocks so the serial tail after the last fold is
    ~1.5us instead of ~5.8us.
"""

import sys

sys.path.insert(0, "/opt/trn_rl_repo")

from contextlib import ExitStack

import numpy as np

import concourse.bass as bass
import concourse.mybir as mybir
import concourse.tile as tile

F32 = mybir.dt.float32
F16 = mybir.dt.float16
BF16 = mybir.dt.bfloat16
U16 = mybir.dt.uint16
ALU = mybir.AluOpType
AXX = mybir.AxisListType.X
EXP = mybir.ActivationFunctionType.Exp

B, D, E = 65536, 1024, 8
N_CORES = 8
B_LOC = B // N_CORES  # 8192
BLK = 512  # rows per block
DC = D // 128  # 8 contraction chunks
PENDING = 6  # matmul stages queued behind the transpose front


def _join(dst: dict, src: dict):
    for k, v in src.items():
        if dst.get(k, -1) < v:
            dst[k] = v


def prune_waits(nc) -> int:
    """Remove semaphore waits that are implied transitively.

    Model: sems are monotone counters (sem-ge-imm waits, sem-inc updates).
    - Engine sequencers evaluate waits in program order, so instruction n+1
      inherits every guarantee instruction n's waits established (G).
    - A satisfied wait (s>=v) implies its producer COMPLETED, which implies
      the producer's own guarantees plus its updates plus (same engine /
      same DMA queue) all earlier completions (C).
    Only sems whose updaters all sit on one engine stream are used as
    evidence (per-queue DMA completion order == trigger order); waits on
    cross-core or mixed-engine sems are never pruned away implicitly and
    never pruned.
    """
    n_pruned = 0
    for f in nc.m.functions:
        for bb in f.blocks:
            insts = bb.instructions
            # per-sem updater order (program order of the updating engine)
            sem_updaters = {}  # sem_id -> [(inst_idx, cum_value)]
            sem_engines = {}
            for i, inst in enumerate(insts):
                si = inst.sync_info
                if not si or not si.on_update:
                    continue
                for u in si.on_update:
                    if u.sync_type != "semaphore" or u.update_mode != "sem-inc":
                        sem_engines[u.id] = "mixed"
                        continue
                    lst = sem_updaters.setdefault(u.id, [])
                    cum = (lst[-1][1] if lst else 0) + (u.update_value or 1)
                    lst.append((i, cum))
                    eng = sem_engines.setdefault(u.id, inst.engine)
                    if eng != inst.engine:
                        sem_engines[u.id] = "mixed"
            orderable = {
                s for s, e in sem_engines.items() if e != "mixed"
            }

            def producer(sem_id, value):
                if sem_id not in orderable:
                    return None
                for i, cum in sem_updaters.get(sem_id, []):
                    if cum >= value:
                        return i
                return None

            # async updaters: sem fires at DMA completion, which races with
            # later instructions on the issuing engine (only same-queue DMA
            # completions are ordered)
            def is_async(inst):
                # DMA triggers complete asynchronously; GpSimd (Pool) is only
                # per-Q7 FIFO, so treat its completions as unordered too.
                tn = type(inst).__name__
                return (
                    "DMA" in tn
                    or "Dma" in tn
                    or inst.engine == mybir.EngineType.Pool
                )

            # own_cum[i] = {sem: cum value} this instruction's updates reach
            own_cum = [dict() for _ in insts]
            queue_pred = [dict() for _ in insts]  # sem -> prev updater idx
            for s, lst in sem_updaters.items():
                if s not in orderable:
                    continue
                prev = None
                for i, cum in lst:
                    own_cum[i][s] = cum
                    if prev is not None:
                        queue_pred[i][s] = prev
                    prev = i

            # G[i]: guarantees when inst i is dispatched (after its waits).
            # C_eng[i]: what the next instruction on the same engine may
            #   assume about i (its dispatch guarantees; plus its own sync
            #   updates only if it is not an async DMA trigger).
            # C_full[i]: what a waiter observing i's sem update may assume
            #   (i completed: dispatch guarantees + same-queue predecessor
            #   completions + own updates).
            G = [None] * len(insts)
            C_eng = [None] * len(insts)
            C_full = [None] * len(insts)
            last_on_engine = {}
            for i, inst in enumerate(insts):
                si = inst.sync_info
                eng = inst.engine
                pred = last_on_engine.get(eng)
                g: dict = dict(G[pred]) if pred is not None else {}
                if si and si.on_wait:
                    waits = [
                        w
                        for w in si.on_wait
                        if w.sync_type == "semaphore"
                        and w.wait_mode == "sem-ge-imm"
                    ]
                    other = [
                        w
                        for w in si.on_wait
                        if not (
                            w.sync_type == "semaphore"
                            and w.wait_mode == "sem-ge-imm"
                        )
                    ]
                    # strongest (latest-producer) waits first
                    def lateness(w):
                        p = producer(w.id, w.wait_value)
                        return -1 if p is None else p

                    waits.sort(key=lateness, reverse=True)
                    kept = []
                    for w in waits:
                        # only monotone single-engine sems may be pruned or
                        # serve as evidence: barrier/event sems can be reset
                        # and reused, so nothing about them persists
                        if w.id not in orderable:
                            kept.append(w)
                            continue
                        if g.get(w.id, -1) >= w.wait_value:
                            n_pruned += 1
                            continue
                        kept.append(w)
                        g[w.id] = w.wait_value
                        p = producer(w.id, w.wait_value)
                        if p is not None and C_full[p] is not None:
                            _join(g, C_full[p])
                    if len(kept) != len(waits):
                        inst.sync_info = mybir.SyncInfo(
                            on_wait=other + kept,
                            on_update=list(si.on_update or []),
                        )
                G[i] = g
                async_i = is_async(inst)
                ce = dict(g)
                if pred is not None:
                    _join(ce, C_eng[pred])
                cf = dict(ce)
                if async_i:
                    cf = dict(g)
                    for s, qp in queue_pred[i].items():
                        if C_full[qp] is not None:
                            _join(cf, C_full[qp])
                for s, cum in own_cum[i].items():
                    if cf.get(s, -1) < cum:
                        cf[s] = cum
                    if not async_i and ce.get(s, -1) < cum:
                        ce[s] = cum
                C_eng[i] = ce
                C_full[i] = cf
                last_on_engine[eng] = i
    return n_pruned


def split_waits(nc, max_waits: int = 1) -> int:
    """walrus allows only one semaphore wait per instruction; hoist the
    rest into preceding single-wait NOPs on the same engine (engine streams
    execute in order, so earlier waits on the same engine are equivalent)."""
    n_split = 0
    for f in nc.m.functions:
        for bb in f.blocks:
            new = []
            for inst in bb.instructions:
                si = inst.sync_info
                if si is not None and si.on_wait and len(si.on_wait) > max_waits:
                    waits = list(si.on_wait)
                    for w in waits[:-max_waits]:
                        n_split += 1
                        nop = mybir.InstNoOp(name=f"{inst.name}-ws{n_split}")
                        nop.engine = inst.engine
                        nop.sync_info = mybir.SyncInfo(on_wait=[w], on_update=[])
                        new.append(nop)
                    inst.sync_info = mybir.SyncInfo(
                        on_wait=waits[-max_waits:], on_update=list(si.on_update or [])
                    )
                new.append(inst)
            bb.instructions = new
    return n_split


def build_module(b_loc: int = B_LOC, split: bool = True):
    assert b_loc % 4096 == 0
    nc = bass.Bass()
    x = nc.dram_tensor("x", [b_loc, D], F32, kind="ExternalInput")
    whl = nc.dram_tensor("whl", [DC, 128, 32], F16, kind="ExternalInput")
    brow = nc.dram_tensor("brow", [512], F32, kind="ExternalInput")
    fold = nc.dram_tensor("fold", [32, 16], F16, kind="ExternalInput")
    ident = nc.dram_tensor("ident", [128, 128], F32, kind="ExternalInput")
    y = nc.dram_tensor("y", [b_loc], F32, kind="ExternalOutput")

    n_blk = b_loc // BLK
    tt = nc.vector.tensor_tensor

    with tile.TileContext(nc) as tc, ExitStack() as ctx:
        consts = ctx.enter_context(tc.tile_pool(name="consts", bufs=1))
        xpool = ctx.enter_context(tc.tile_pool(name="xpool", bufs=3))
        xh_pool = ctx.enter_context(tc.tile_pool(name="xh", bufs=8))
        xl_pool = ctx.enter_context(tc.tile_pool(name="xl", bufs=8))
        z32_pool = ctx.enter_context(tc.tile_pool(name="z32", bufs=3))
        pp = ctx.enter_context(tc.tile_pool(name="pp", bufs=2))
        ov_pool = ctx.enter_context(tc.tile_pool(name="ov", bufs=2))
        xt_pool = ctx.enter_context(tc.tile_pool(name="xtps", bufs=5, space="PSUM"))
        z_pool = ctx.enter_context(tc.tile_pool(name="zps", bufs=2, space="PSUM"))
        zt_pool = ctx.enter_context(tc.tile_pool(name="ztps", bufs=1, space="PSUM"))

        ident_sb = consts.tile([128, 128], F32)
        whl_sb = consts.tile([128, DC, 32], F16)
        fold_sb = consts.tile([32, 16], F16)
        bias_sb = consts.tile([128, 512], F32)

        def pp_part(zt_ps, pair_i, outv):
            """Elementwise/reduce postprocess of 2 blocks = cols
            [128*pair_i : 128*pair_i+128) of zt_ps -> outv[:, 8p:8p+8]."""
            c0 = 128 * pair_i
            zsl = zt_ps[:, c0 : c0 + 128].rearrange("p (g e) -> p g e", e=16)
            bsl = bias_sb[:, c0 : c0 + 128].rearrange("p (g e) -> p g e", e=16)
            zb = pp.tile([128, 8, 16], F32)
            nc.vector.tensor_add(zb, zsl, bsl)
            g8 = zb[:, :, 0:8]
            y8 = zb[:, :, 8:16]
            p8 = pp.tile([128, 8, 8], F32)
            nc.scalar.activation(p8, g8, EXP)
            den = pp.tile([128, 8], F32)
            nc.vector.tensor_reduce(den, p8, axis=AXX, op=ALU.add)
            # top-2 threshold: tournament keeping (max, 2nd max) per segment
            h1 = pp.tile([128, 8, 4], F32)
            l1 = pp.tile([128, 8, 4], F32)
            tt(h1, p8[:, :, 0:4], p8[:, :, 4:8], op=ALU.max)
            tt(l1, p8[:, :, 0:4], p8[:, :, 4:8], op=ALU.min)
            h2 = pp.tile([128, 8, 2], F32)
            v2 = pp.tile([128, 8, 2], F32)
            u2 = pp.tile([128, 8, 2], F32)
            m2q = pp.tile([128, 8, 2], F32)
            tt(h2, h1[:, :, 0:2], h1[:, :, 2:4], op=ALU.max)
            tt(v2, h1[:, :, 0:2], h1[:, :, 2:4], op=ALU.min)
            tt(u2, l1[:, :, 0:2], l1[:, :, 2:4], op=ALU.max)
            tt(m2q, u2, v2, op=ALU.max)
            v3 = pp.tile([128, 8, 1], F32)
            u3 = pp.tile([128, 8, 1], F32)
            m2f = pp.tile([128, 8, 1], F32)
            tt(v3, h2[:, :, 0:1], h2[:, :, 1:2], op=ALU.min)
            tt(u3, m2q[:, :, 0:1], m2q[:, :, 1:2], op=ALU.max)
            tt(m2f, u3, v3, op=ALU.max)
            # mask & weighted sum
            msk = pp.tile([128, 8, 8], F32)
            tt(msk, p8, m2f.to_broadcast([128, 8, 8]), op=ALU.is_ge)
            pm = pp.tile([128, 8, 8], F32)
            tt(pm, msk, p8, op=ALU.mult)
            prod = pp.tile([128, 8, 8], F32)
            tt(prod, pm, y8, op=ALU.mult)
            num = pp.tile([128, 8], F32)
            nc.vector.tensor_reduce(num, prod, axis=AXX, op=ALU.add)
            rden = pp.tile([128, 8], F32)
            nc.vector.reciprocal(rden, den)
            tt(outv[:, 8 * pair_i : 8 * pair_i + 8], num, rden, op=ALU.mult)

        def pp_finish(outv, b0):
            # 32x32 block transpose so each partition holds one contiguous run
            tv = pp.tile([128, 32], F32)
            nc.vector.transpose(tv, outv)
            yf = y.ap()
            for a in range(4):
                # dest[i, k] = y[b0 + 32a + 128 i + k], i,k in 0..32
                dest = bass.AP(yf.tensor, b0 + 32 * a, [[128, 32], [1, 32]])
                nc.sync.dma_start(out=dest, in_=tv[32 * a : 32 * a + 32, :])

        state = {"zt_ps": None, "outv": None}

        def emit_stage(blk, c, z_ps, xh, xl):
            # hi stream (bf16 view) and lo stream (fp16), both against the
            # full [w_hi|w_lo] fp16 pair: together they accumulate the
            # complete product (hi+lo)*(w_hi+w_lo) into rows 0:32.
            nc.tensor.matmul(
                z_ps, whl_sb[:, c, :], xh.bitcast(BF16),
                start=(c == 0), stop=False,
            )
            nc.tensor.matmul(
                z_ps, whl_sb[:, c, :], xl, start=False, stop=(c == DC - 1)
            )
            if c == DC - 1:
                # block tail: scores as an exact fp16 (hi, lo) pair so the
                # fold matmuls get single-pass fp16 weight loads
                zz = z32_pool.tile([32, 1024], F16)
                nc.scalar.copy(zz[:, 0:512], z_ps)
                nc.vector.tensor_sub(zz[:, 512:1024], z_ps, zz[:, 0:512])
                bank_i = blk % 8
                if bank_i == 0:
                    state["zt_ps"] = zt_pool.tile(
                        [128, 512], F32, name="zt_ps", tag="zt_ps"
                    )
                    state["outv"] = ov_pool.tile(
                        [128, 32], F32, name="outv", tag="outv"
                    )
                zt_ps = state["zt_ps"]
                for j in range(4):
                    col = (bank_i * 4 + j) * 16
                    nc.tensor.matmul(
                        zt_ps[:, col : col + 16],
                        zz[:, 128 * j : 128 * j + 128],
                        fold_sb,
                        start=True,
                        stop=False,
                    )
                    nc.tensor.matmul(
                        zt_ps[:, col : col + 16],
                        zz[:, 512 + 128 * j : 512 + 128 * j + 128],
                        fold_sb,
                        start=False,
                        stop=True,
                    )
                if bank_i % 2 == 1:
                    psave = tc.cur_priority
                    tc.cur_priority = psave + 100000
                    pp_part(zt_ps, bank_i // 2, state["outv"])
                    if bank_i == 7:
                        pp_finish(state["outv"], (blk // 8) * 4096)
                    tc.cur_priority = psave

        pending = []
        for blk in range(n_blk):
            r0 = blk * BLK
            xin = x.ap()[r0 : r0 + BLK, :].rearrange("(j p) d -> p j d", p=128)
            # one tile per D-quarter so a chunk's transposes only wait for
            # their own 512KB; block 0 splits the first quarter into per-chunk
            # tiles so the very first transpose starts as early as possible
            if blk == 0:
                xp_a = xpool.tile([128, 4, 128], F32, name="xp_a")
                xp_b = xpool.tile([128, 4, 128], F32, name="xp_b")
                xp_c = xpool.tile([128, 4, 256], F32, name="xp_c")
                nc.sync.dma_start(out=ident_sb, in_=ident.ap())
                nc.sync.dma_start(out=xp_a, in_=xin[:, :, 0:128])
                nc.scalar.dma_start(out=whl_sb, in_=whl.ap().transpose([1, 0, 2]))
                nc.sync.dma_start(out=xp_b, in_=xin[:, :, 128:256])
                nc.scalar.dma_start(out=fold_sb, in_=fold.ap())
                nc.gpsimd.dma_start(
                    out=bias_sb,
                    in_=brow.ap().unsqueeze(0).to_broadcast([128, 512]),
                )
                nc.sync.dma_start(out=xp_c, in_=xin[:, :, 256:512])
                q23 = []
                for q in (2, 3):
                    t = xpool.tile([128, 4, 256], F32, name=f"xq{q}")
                    nc.sync.dma_start(
                        out=t, in_=xin[:, :, 256 * q : 256 * q + 256]
                    )
                    q23.append(t)

                def srcmap(c):
                    if c == 0:
                        return xp_a, 0
                    if c == 1:
                        return xp_b, 0
                    if c < 4:
                        return xp_c, 128 * (c - 2)
                    return q23[c // 2 - 2], 128 * (c % 2)
            else:
                xq = []
                for q in range(4):
                    t = xpool.tile([128, 4, 256], F32, name=f"xq{q}")
                    nc.sync.dma_start(
                        out=t, in_=xin[:, :, 256 * q : 256 * q + 256]
                    )
                    xq.append(t)

                def srcmap(c, xq=xq):
                    return xq[c // 2], 128 * (c % 2)
            z_ps = z_pool.tile([32, 512], F32)
            # batch 4 chunks of transposes, then that many matmul stages:
            # transpose-mode <-> matmul-mode switches cost ~100ns each on
            # the PE, so alternating per chunk wastes ~25us/core
            for g in range(DC // 4):
                for c in range(4 * g, 4 * g + 4):
                    xt_ps = xt_pool.tile([128, 512], F32)
                    src, d0 = srcmap(c)
                    for j in range(4):
                        nc.tensor.transpose(
                            xt_ps[:, 128 * j : 128 * j + 128],
                            src[:, j, d0 : d0 + 128],
                            ident_sb,
                        )
                    # hi = bf16 truncation of xT: u16 bit-slice copy on ACT
                    # (psum -> sbuf); lo = fp16(xT - hi) on DVE.
                    xt_hi_view = (
                        xt_ps.bitcast(U16)
                        .rearrange("p (k two) -> p k two", two=2)[:, :, 1]
                    )
                    xh = xh_pool.tile([128, 512], U16)
                    nc.scalar.copy(xh, xt_hi_view)
                    xl = xl_pool.tile([128, 512], F16)
                    nc.vector.tensor_sub(xl, xt_ps, xh.bitcast(BF16))
                    pending.append((blk, c, z_ps, xh, xl))
                while len(pending) > 4:
                    emit_stage(*pending.pop(0))
        for args in pending:
            emit_stage(*args)

    if split:
        import os

        n_pruned = 0
        if not os.environ.get("MOE_NO_PRUNE"):
            n_pruned = prune_waits(nc)
        n_split = split_waits(nc)
        if os.environ.get("MOE_DEBUG"):
            print(f"prune_waits: {n_pruned} pruned; split_waits: {n_split} NOPs")
    return nc


def host_inputs(gate_w, gate_b, expert_w, expert_b):
    """Host-side prep of the small replicated tensors."""
    W = np.concatenate([gate_w, expert_w], axis=0).astype(np.float32)  # [16, D]
    WT = W.T  # [D, 16]
    w_hi = WT.astype(np.float16)
    w_lo = (WT - w_hi.astype(np.float32)).astype(np.float16)
    whl = np.empty((DC, 128, 32), dtype=np.float16)
    for c in range(DC):
        whl[c, :, 0:16] = w_hi[128 * c : 128 * (c + 1), :]
        whl[c, :, 16:32] = w_lo[128 * c : 128 * (c + 1), :]
    bcat = np.concatenate([gate_b, expert_b]).astype(np.float32)  # [16]
    brow = np.tile(bcat, 32)  # [512]
    fold = np.concatenate([np.eye(16), np.eye(16)], axis=0).astype(np.float16)
    ident = np.eye(128, dtype=np.float32)
    return {"whl": whl, "brow": brow, "fold": fold, "ident": ident}


_NC_CACHE = {}


def kernel(x, gate_w, gate_b, expert_w, expert_b, k):
    assert int(k) == 2
    x = np.ascontiguousarray(np.asarray(x, dtype=np.float32))
    assert x.shape == (B, D)

    from concourse.bass_utils import run_bass_kernel_spmd

    if B_LOC not in _NC_CACHE:
        _NC_CACHE[B_LOC] = build_module(B_LOC)
    nc = _NC_CACHE[B_LOC]

    common = host_inputs(
        np.asarray(gate_w, np.float32),
        np.asarray(gate_b, np.float32),
        np.asarray(expert_w, np.float32),
        np.asarray(expert_b, np.float32),
    )
    in_maps = [
        {**common, "x": x[i * B_LOC : (i + 1) * B_LOC]} for i in range(N_CORES)
    ]
    import os

    trace = bool(os.environ.get("MOE_TRACE"))
    if trace:
        _ensure_ntff_hook()
    res = run_bass_kernel_spmd(
        nc, in_maps, core_ids=list(range(N_CORES)), trace=trace
    )
    global LAST_RESULT
    LAST_RESULT = res
    out = np.concatenate([r["y"] for r in res.results])
    return out.reshape(B, 1).astype(np.float32)


LAST_RESULT = None


def _ensure_ntff_hook():
    """Register the axon NTFF profile hook if the antenv shim is missing
    (lets run_bass_kernel_spmd(trace=True) capture HW timing)."""
    try:
        import antenv.axon_hooks  # noqa: F401

        return
    except ImportError:
        pass
    try:
        import types

        import antenv
        from trn_agent_boot.trn_boot import _ntff_profile_via_ctypes

        mod = types.ModuleType("antenv.axon_hooks")
        _h = [None]
        mod.set_axon_ntff_profile_hook = lambda h: _h.__setitem__(0, h)
        mod.get_axon_ntff_profile_hook = lambda: _h[0]
        sys.modules["antenv.axon_hooks"] = mod
        antenv.axon_hooks = mod
        mod.set_axon_ntff_profile_hook(
            _ntff_profile_via_ctypes("/opt/axon/libaxon_pjrt.so")
        )
    except Exception as e:  # profiling is best-effort
        print(f"ntff hook setup failed: {e}")


if __name__ == "__main__":
    rng = np.random.default_rng(0)
    s = 1.0 / np.sqrt(D)
    inputs = {
        "x": rng.standard_normal((B, D), dtype=np.float32),
        "gate_w": rng.uniform(-s, s, (E, D)).astype(np.float32),
        "gate_b": rng.uniform(-s, s, E).astype(np.float32),
        "expert_w": rng.uniform(-s, s, (E, D)).astype(np.float32),
        "expert_b": rng.uniform(-s, s, E).astype(np.float32),
        "k": 2,
    }
    got = kernel(**inputs)
    print("kernel output:", got.shape, got.dtype, got[:4, 0])


# revision 12
# speedup vs baseline: 1.0227x; 1.0066x over previous
"""Trainium2 Bass kernel for nn_MoEGate_6150393168540 (moe_routing).

Computes, for x [B=65536, D=1024], gate/expert weights [E=8, D] and biases [E]:
    gate = softmax(x @ gate_w.T + gate_b)            # [B, 8]
    keep top-k (k=2) gate values, zero the rest (no renormalization)
    expert = x @ expert_w.T + expert_b               # [B, 8]
    out = sum(gate_masked * expert, axis=1)          # [B, 1]

Strategy (8 NeuronCores, data-parallel over the batch):
  - Each core gets 8192 rows of x; weights are replicated.
  - The D-contraction needs x transposed (D on partitions). PE transposes x in
    fp32 ([128,128] blocks -> PSUM, bit-exact, 2 cyc/row).
  - Scores must order-match a pure-fp32 reference (top-2 selection), so the
    matmul uses an exact fp16 Dekker split: hi = bf16-trunc(xT) (ACT bit-slice
    copy from PSUM), lo = fp16(xT - hi) (DVE). Weights are split host-side the
    same way. hi*w_hi, hi*w_lo, lo*w_hi are exact products accumulated in fp32
    PSUM => scores accurate to ~1e-7, at 16-bit matmul speed.
  - mm_hi: lhsT=[w_hi|w_lo] [128,32] -> psum rows 0:32; mm_lo accumulates.
  - A tiny "fold" matmul (lhsT = scores-pair fp16, rhs = [I16;I16]) both
    transposes scores back to batch-major and sums hi+lo partial rows.
  - Postprocess per 2 blocks (slice of the 8-block PSUM bank): +bias, exp on
    ACT, top-2 threshold via min/max tournament, mask, weighted sum, divide.
    A final DVE 32x32 transpose per 8-block group makes the output DMA
    contiguous.

Perf notes (vs the 181us first version):
  - x is DMAed in D-quarters with one tile per quarter so the first transpose
    only waits ~512KB, not the full 2MB block (head -8us).
  - prune_waits() removes semaphore waits implied by engine issue order or by
    the transitive closure of a producer's own waits; walrus only supports one
    wait per instruction, so fewer waits means fewer NOPs and less sem-eval
    latency per PE/ACT/DVE instruction.
  - postprocess runs per 2 blocks so the serial tail after the last fold is
    ~1.5us instead of ~5.8us.
"""

import sys

sys.path.insert(0, "/opt/trn_rl_repo")

from contextlib import ExitStack

import numpy as np

import concourse.bass as bass
import concourse.mybir as mybir
import concourse.tile as tile

F32 = mybir.dt.float32
F16 = mybir.dt.float16
BF16 = mybir.dt.bfloat16
U16 = mybir.dt.uint16
ALU = mybir.AluOpType
AXX = mybir.AxisListType.X
EXP = mybir.ActivationFunctionType.Exp

B, D, E = 65536, 1024, 8
N_CORES = 8
B_LOC = B // N_CORES  # 8192
BLK = 512  # rows per block
DC = D // 128  # 8 contraction chunks
PENDING = 6  # matmul stages queued behind the transpose front


def _join(dst: dict, src: dict):
    for k, v in src.items():
        if dst.get(k, -1) < v:
            dst[k] = v


def prune_waits(nc) -> int:
    """Remove semaphore waits that are implied transitively.

    Model: sems are monotone counters (sem-ge-imm waits, sem-inc updates).
    - Engine sequencers evaluate waits in program order, so instruction n+1
      inherits every guarantee instruction n's waits established (G).
    - A satisfied wait (s>=v) implies its producer COMPLETED, which implies
      the producer's own guarantees plus its updates plus (same engine /
      same DMA queue) all earlier completions (C).
    Only sems whose updaters all sit on one engine stream are used as
    evidence (per-queue DMA completion order == trigger order); waits on
    cross-core or mixed-engine sems are never pruned away implicitly and
    never pruned.
    """
    n_pruned = 0
    for f in nc.m.functions:
        for bb in f.blocks:
            insts = bb.instructions
            # per-sem updater order (program order of the updating engine)
            sem_updaters = {}  # sem_id -> [(inst_idx, cum_value)]
            sem_engines = {}
            for i, inst in enumerate(insts):
                si = inst.sync_info
                if not si or not si.on_update:
                    continue
                for u in si.on_update:
                    if u.sync_type != "semaphore" or u.update_mode != "sem-inc":
                        sem_engines[u.id] = "mixed"
                        continue
                    lst = sem_updaters.setdefault(u.id, [])
                    cum = (lst[-1][1] if lst else 0) + (u.update_value or 1)
                    lst.append((i, cum))
                    eng = sem_engines.setdefault(u.id, inst.engine)
                    if eng != inst.engine:
                        sem_engines[u.id] = "mixed"
            orderable = {
                s for s, e in sem_engines.items() if e != "mixed"
            }

            def producer(sem_id, value):
                if sem_id not in orderable:
                    return None
                for i, cum in sem_updaters.get(sem_id, []):
                    if cum >= value:
                        return i
                return None

            # async updaters: sem fires at DMA completion, which races with
            # later instructions on the issuing engine (only same-queue DMA
            # completions are ordered)
            def is_async(inst):
                # DMA triggers complete asynchronously; GpSimd (Pool) is only
                # per-Q7 FIFO, so treat its completions as unordered too.
                tn = type(inst).__name__
                return (
                    "DMA" in tn
                    or "Dma" in tn
                    or inst.engine == mybir.EngineType.Pool
                )

            # own_cum[i] = {sem: cum value} this instruction's updates reach
            own_cum = [dict() for _ in insts]
            queue_pred = [dict() for _ in insts]  # sem -> prev updater idx
            for s, lst in sem_updaters.items():
                if s not in orderable:
                    continue
                prev = None
                for i, cum in lst:
                    own_cum[i][s] = cum
                    if prev is not None:
                        queue_pred[i][s] = prev
                    prev = i

            # G[i]: guarantees when inst i is dispatched (after its waits).
            # C_eng[i]: what the next instruction on the same engine may
            #   assume about i (its dispatch guarantees; plus its own sync
            #   updates only if it is not an async DMA trigger).
            # C_full[i]: what a waiter observing i's sem update may assume
            #   (i completed: dispatch guarantees + same-queue predecessor
            #   completions + own updates).
            G = [None] * len(insts)
            C_eng = [None] * len(insts)
            C_full = [None] * len(insts)
            last_on_engine = {}
            for i, inst in enumerate(insts):
                si = inst.sync_info
                eng = inst.engine
                pred = last_on_engine.get(eng)
                g: dict = dict(G[pred]) if pred is not None else {}
                if si and si.on_wait:
                    waits = [
                        w
                        for w in si.on_wait
                        if w.sync_type == "semaphore"
                        and w.wait_mode == "sem-ge-imm"
                    ]
                    other = [
                        w
                        for w in si.on_wait
                        if not (
                            w.sync_type == "semaphore"
                            and w.wait_mode == "sem-ge-imm"
                        )
                    ]
                    # strongest (latest-producer) waits first
                    def lateness(w):
                        p = producer(w.id, w.wait_value)
                        return -1 if p is None else p

                    waits.sort(key=lateness, reverse=True)
                    kept = []
                    for w in waits:
                        # only monotone single-engine sems may be pruned or
                        # serve as evidence: barrier/event sems can be reset
                        # and reused, so nothing about them persists
                        if w.id not in orderable:
                            kept.append(w)
                            continue
                        if g.get(w.id, -1) >= w.wait_value:
                            n_pruned += 1
                            continue
                        kept.append(w)
                        g[w.id] = w.wait_value
                        p = producer(w.id, w.wait_value)
                        if p is not None and C_full[p] is not None:
                            _join(g, C_full[p])
                    if len(kept) != len(waits):
                        inst.sync_info = mybir.SyncInfo(
                            on_wait=other + kept,
                            on_update=list(si.on_update or []),
                        )
                G[i] = g
                async_i = is_async(inst)
                ce = dict(g)
                if pred is not None:
                    _join(ce, C_eng[pred])
                cf = dict(ce)
                if async_i:
                    cf = dict(g)
                    for s, qp in queue_pred[i].items():
                        if C_full[qp] is not None:
                            _join(cf, C_full[qp])
                for s, cum in own_cum[i].items():
                    if cf.get(s, -1) < cum:
                        cf[s] = cum
                    if not async_i and ce.get(s, -1) < cum:
                        ce[s] = cum
                C_eng[i] = ce
                C_full[i] = cf
                last_on_engine[eng] = i
    return n_pruned


def split_waits(nc, max_waits: int = 1) -> int:
    """walrus allows only one semaphore wait per instruction; hoist the
    rest into preceding single-wait NOPs on the same engine (engine streams
    execute in order, so earlier waits on the same engine are equivalent)."""
    n_split = 0
    for f in nc.m.functions:
        for bb in f.blocks:
            new = []
            for inst in bb.instructions:
                si = inst.sync_info
                if si is not None and si.on_wait and len(si.on_wait) > max_waits:
                    waits = list(si.on_wait)
                    for w in waits[:-max_waits]:
                        n_split += 1
                        nop = mybir.InstNoOp(name=f"{inst.name}-ws{n_split}")
                        nop.engine = inst.engine
                        nop.sync_info = mybir.SyncInfo(on_wait=[w], on_update=[])
                        new.append(nop)
                    inst.sync_info = mybir.SyncInfo(
                        on_wait=waits[-max_waits:], on_update=list(si.on_update or [])
                    )
                new.append(inst)
            bb.instructions = new
    return n_split


def build_module(b_loc: int = B_LOC, split: bool = True):
    assert b_loc % 4096 == 0
    nc = bass.Bass()
    x = nc.dram_tensor("x", [b_loc, D], F32, kind="ExternalInput")
    whl = nc.dram_tensor("whl", [DC, 128, 32], F16, kind="ExternalInput")
    brow = nc.dram_tensor("brow", [512], F32, kind="ExternalInput")
    fold = nc.dram_tensor("fold", [32, 16], F16, kind="ExternalInput")
    ident = nc.dram_tensor("ident", [128, 128], F32, kind="ExternalInput")
    y = nc.dram_tensor("y", [b_loc], F32, kind="ExternalOutput")

    n_blk = b_loc // BLK
    tt = nc.vector.tensor_tensor

    with tile.TileContext(nc) as tc, ExitStack() as ctx:
        consts = ctx.enter_context(tc.tile_pool(name="consts", bufs=1))
        xpool = ctx.enter_context(tc.tile_pool(name="xpool", bufs=4))
        xh_pool = ctx.enter_context(tc.tile_pool(name="xh", bufs=8))
        xl_pool = ctx.enter_context(tc.tile_pool(name="xl", bufs=8))
        z32_pool = ctx.enter_context(tc.tile_pool(name="z32", bufs=3))
        pp = ctx.enter_context(tc.tile_pool(name="pp", bufs=2))
        ov_pool = ctx.enter_context(tc.tile_pool(name="ov", bufs=2))
        xt_pool = ctx.enter_context(tc.tile_pool(name="xtps", bufs=5, space="PSUM"))
        z_pool = ctx.enter_context(tc.tile_pool(name="zps", bufs=2, space="PSUM"))
        zt_pool = ctx.enter_context(tc.tile_pool(name="ztps", bufs=1, space="PSUM"))

        ident_sb = consts.tile([128, 128], F32)
        whl_sb = consts.tile([128, DC, 32], F16)
        fold_sb = consts.tile([32, 16], F16)
        bias_sb = consts.tile([128, 512], F32)

        def pp_part(zt_ps, pair_i, outv):
            """Elementwise/reduce postprocess of 2 blocks = cols
            [128*pair_i : 128*pair_i+128) of zt_ps -> outv[:, 8p:8p+8]."""
            c0 = 128 * pair_i
            zsl = zt_ps[:, c0 : c0 + 128].rearrange("p (g e) -> p g e", e=16)
            bsl = bias_sb[:, c0 : c0 + 128].rearrange("p (g e) -> p g e", e=16)
            zb = pp.tile([128, 8, 16], F32)
            nc.vector.tensor_add(zb, zsl, bsl)
            g8 = zb[:, :, 0:8]
            y8 = zb[:, :, 8:16]
            p8 = pp.tile([128, 8, 8], F32)
            nc.scalar.activation(p8, g8, EXP)
            den = pp.tile([128, 8], F32)
            nc.vector.tensor_reduce(den, p8, axis=AXX, op=ALU.add)
            # top-2 threshold: tournament keeping (max, 2nd max) per segment
            h1 = pp.tile([128, 8, 4], F32)
            l1 = pp.tile([128, 8, 4], F32)
            tt(h1, p8[:, :, 0:4], p8[:, :, 4:8], op=ALU.max)
            tt(l1, p8[:, :, 0:4], p8[:, :, 4:8], op=ALU.min)
            h2 = pp.tile([128, 8, 2], F32)
            v2 = pp.tile([128, 8, 2], F32)
            u2 = pp.tile([128, 8, 2], F32)
            m2q = pp.tile([128, 8, 2], F32)
            tt(h2, h1[:, :, 0:2], h1[:, :, 2:4], op=ALU.max)
            tt(v2, h1[:, :, 0:2], h1[:, :, 2:4], op=ALU.min)
            tt(u2, l1[:, :, 0:2], l1[:, :, 2:4], op=ALU.max)
            tt(m2q, u2, v2, op=ALU.max)
            v3 = pp.tile([128, 8, 1], F32)
            u3 = pp.tile([128, 8, 1], F32)
            m2f = pp.tile([128, 8, 1], F32)
            tt(v3, h2[:, :, 0:1], h2[:, :, 1:2], op=ALU.min)
            tt(u3, m2q[:, :, 0:1], m2q[:, :, 1:2], op=ALU.max)
            tt(m2f, u3, v3, op=ALU.max)
            # mask & weighted sum
            msk = pp.tile([128, 8, 8], F32)
            tt(msk, p8, m2f.to_broadcast([128, 8, 8]), op=ALU.is_ge)
            pm = pp.tile([128, 8, 8], F32)
            tt(pm, msk, p8, op=ALU.mult)
            prod = pp.tile([128, 8, 8], F32)
            tt(prod, pm, y8, op=ALU.mult)
            num = pp.tile([128, 8], F32)
            nc.vector.tensor_reduce(num, prod, axis=AXX, op=ALU.add)
            rden = pp.tile([128, 8], F32)
            nc.vector.reciprocal(rden, den)
            tt(outv[:, 8 * pair_i : 8 * pair_i + 8], num, rden, op=ALU.mult)

        def pp_finish(outv, b0, last=False):
            # 32x32 block transpose so each partition holds one contiguous run
            tv = pp.tile([128, 32], F32)
            nc.vector.transpose(tv, outv)
            yf = y.ap()
            for a in range(4):
                # dest[i, k] = y[b0 + 32a + 128 i + k], i,k in 0..32
                dest = bass.AP(yf.tensor, b0 + 32 * a, [[128, 32], [1, 32]])
                eng = nc.scalar if (last and a >= 2) else nc.sync
                eng.dma_start(out=dest, in_=tv[32 * a : 32 * a + 32, :])

        state = {"zt_ps": None, "outv": None}

        def emit_stage(blk, c, z_ps, xh, xl):
            # hi stream (bf16 view) and lo stream (fp16), both against the
            # full [w_hi|w_lo] fp16 pair: together they accumulate the
            # complete product (hi+lo)*(w_hi+w_lo) into rows 0:32.
            nc.tensor.matmul(
                z_ps, whl_sb[:, c, :], xh.bitcast(BF16),
                start=(c == 0), stop=False,
            )
            nc.tensor.matmul(
                z_ps, whl_sb[:, c, :], xl, start=False, stop=(c == DC - 1)
            )
            if c == DC - 1:
                # block tail: scores as an exact fp16 (hi, lo) pair so the
                # fold matmuls get single-pass fp16 weight loads
                zz = z32_pool.tile([32, 1024], F16)
                nc.scalar.copy(zz[:, 0:512], z_ps)
                nc.vector.tensor_sub(zz[:, 512:1024], z_ps, zz[:, 0:512])
                bank_i = blk % 8
                if bank_i == 0:
                    state["zt_ps"] = zt_pool.tile(
                        [128, 512], F32, name="zt_ps", tag="zt_ps"
                    )
                    state["outv"] = ov_pool.tile(
                        [128, 32], F32, name="outv", tag="outv"
                    )
                zt_ps = state["zt_ps"]
                for j in range(4):
                    col = (bank_i * 4 + j) * 16
                    nc.tensor.matmul(
                        zt_ps[:, col : col + 16],
                        zz[:, 128 * j : 128 * j + 128],
                        fold_sb,
                        start=True,
                        stop=False,
                    )
                    nc.tensor.matmul(
                        zt_ps[:, col : col + 16],
                        zz[:, 512 + 128 * j : 512 + 128 * j + 128],
                        fold_sb,
                        start=False,
                        stop=True,
                    )
                if bank_i % 2 == 1:
                    last_grp = blk // 8 == n_blk // 8 - 1
                    psave = tc.cur_priority
                    if not last_grp:
                        tc.cur_priority = psave + 100000
                    pp_part(zt_ps, bank_i // 2, state["outv"])
                    if bank_i == 7:
                        pp_finish(state["outv"], (blk // 8) * 4096, last=last_grp)
                    tc.cur_priority = psave

        pending = []
        for blk in range(n_blk):
            r0 = blk * BLK
            xin = x.ap()[r0 : r0 + BLK, :].rearrange("(j p) d -> p j d", p=128)
            # one tile per D-quarter so a chunk's transposes only wait for
            # their own 512KB; block 0 splits the first quarter into per-chunk
            # tiles so the very first transpose starts as early as possible
            xq = []
            for q in range(4):
                xq.append(xpool.tile([128, 4, 256], F32, name=f"xq{q}"))
            if blk == 0:
                nc.sync.dma_start(out=ident_sb, in_=ident.ap())
                nc.sync.dma_start(out=xq[0], in_=xin[:, :, 0:256])
                nc.scalar.dma_start(out=whl_sb, in_=whl.ap().transpose([1, 0, 2]))
                nc.scalar.dma_start(out=fold_sb, in_=fold.ap())
                nc.gpsimd.dma_start(
                    out=bias_sb,
                    in_=brow.ap().unsqueeze(0).to_broadcast([128, 512]),
                )
                for q in range(1, 4):
                    nc.sync.dma_start(
                        out=xq[q], in_=xin[:, :, 256 * q : 256 * q + 256]
                    )
            else:
                for q in range(4):
                    nc.sync.dma_start(
                        out=xq[q], in_=xin[:, :, 256 * q : 256 * q + 256]
                    )

            def srcmap(c, xq=xq):
                return xq[c // 2], 128 * (c % 2)
            z_ps = z_pool.tile([32, 512], F32)
            # batch 4 chunks of transposes, then that many matmul stages:
            # transpose-mode <-> matmul-mode switches cost ~100ns each on
            # the PE, so alternating per chunk wastes ~25us/core
            for g in range(DC // 4):
                for c in range(4 * g, 4 * g + 4):
                    xt_ps = xt_pool.tile([128, 512], F32)
                    src, d0 = srcmap(c)
                    for j in range(4):
                        nc.tensor.transpose(
                            xt_ps[:, 128 * j : 128 * j + 128],
                            src[:, j, d0 : d0 + 128],
                            ident_sb,
                        )
                    # hi = bf16 truncation of xT: u16 bit-slice copy on ACT
                    # (psum -> sbuf); lo = fp16(xT - hi) on DVE.
                    xt_hi_view = (
                        xt_ps.bitcast(U16)
                        .rearrange("p (k two) -> p k two", two=2)[:, :, 1]
                    )
                    xh = xh_pool.tile([128, 512], U16)
                    nc.scalar.copy(xh, xt_hi_view)
                    xl = xl_pool.tile([128, 512], F16)
                    nc.vector.tensor_sub(xl, xt_ps, xh.bitcast(BF16))
                    pending.append((blk, c, z_ps, xh, xl))
                while len(pending) > 4:
                    emit_stage(*pending.pop(0))
        for args in pending:
            emit_stage(*args)

    if split:
        import os

        n_pruned = 0
        if not os.environ.get("MOE_NO_PRUNE"):
            n_pruned = prune_waits(nc)
        n_split = split_waits(nc)
        if os.environ.get("MOE_DEBUG"):
            print(f"prune_waits: {n_pruned} pruned; split_waits: {n_split} NOPs")
    return nc


def host_inputs(gate_w, gate_b, expert_w, expert_b):
    """Host-side prep of the small replicated tensors."""
    W = np.concatenate([gate_w, expert_w], axis=0).astype(np.float32)  # [16, D]
    WT = W.T  # [D, 16]
    w_hi = WT.astype(np.float16)
    w_lo = (WT - w_hi.astype(np.float32)).astype(np.float16)
    whl = np.empty((DC, 128, 32), dtype=np.float16)
    for c in range(DC):
        whl[c, :, 0:16] = w_hi[128 * c : 128 * (c + 1), :]
        whl[c, :, 16:32] = w_lo[128 * c : 128 * (c + 1), :]
    bcat = np.concatenate([gate_b, expert_b]).astype(np.float32)  # [16]
    brow = np.tile(bcat, 32)  # [512]
    fold = np.concatenate([np.eye(16), np.eye(16)], axis=0).astype(np.float16)
    ident = np.eye(128, dtype=np.float32)
    return {"whl": whl, "brow": brow, "fold": fold, "ident": ident}


_NC_CACHE = {}


def kernel(x, gate_w, gate_b, expert_w, expert_b, k):
    assert int(k) == 2
    x = np.ascontiguousarray(np.asarray(x, dtype=np.float32))
    assert x.shape == (B, D)

    from concourse.bass_utils import run_bass_kernel_spmd

    if B_LOC not in _NC_CACHE:
        _NC_CACHE[B_LOC] = build_module(B_LOC)
    nc = _NC_CACHE[B_LOC]

    common = host_inputs(
        np.asarray(gate_w, np.float32),
        np.asarray(gate_b, np.float32),
        np.asarray(expert_w, np.float32),
        np.asarray(expert_b, np.float32),
    )
    in_maps = [
        {**common, "x": x[i * B_LOC : (i + 1) * B_LOC]} for i in range(N_CORES)
    ]
    import os

    trace = bool(os.environ.get("MOE_TRACE"))
    if trace:
        _ensure_ntff_hook()
    res = run_bass_kernel_spmd(
        nc, in_maps, core_ids=list(range(N_CORES)), trace=trace
    )
    global LAST_RESULT
    LAST_RESULT = res
    out = np.concatenate([r["y"] for r in res.results])
    return out.reshape(B, 1).astype(np.float32)


LAST_RESULT = None


def _ensure_ntff_hook():
    """Register the axon NTFF profile hook if the antenv shim is missing
    (lets run_bass_kernel_spmd(trace=True) capture HW timing)."""
    try:
        import antenv.axon_hooks  # noqa: F401

        return
    except ImportError:
        pass
    try:
        import types

        import antenv
        from trn_agent_boot.trn_boot import _ntff_profile_via_ctypes

        mod = types.ModuleType("antenv.axon_hooks")
        _h = [None]
        mod.set_axon_ntff_profile_hook = lambda h: _h.__setitem__(0, h)
        mod.get_axon_ntff_profile_hook = lambda: _h[0]
        sys.modules["antenv.axon_hooks"] = mod
        antenv.axon_hooks = mod
        mod.set_axon_ntff_profile_hook(
            _ntff_profile_via_ctypes("/opt/axon/libaxon_pjrt.so")
        )
    except Exception as e:  # profiling is best-effort
        print(f"ntff hook setup failed: {e}")


if __name__ == "__main__":
    rng = np.random.default_rng(0)
    s = 1.0 / np.sqrt(D)
    inputs = {
        "x": rng.standard_normal((B, D), dtype=np.float32),
        "gate_w": rng.uniform(-s, s, (E, D)).astype(np.float32),
        "gate_b": rng.uniform(-s, s, E).astype(np.float32),
        "expert_w": rng.uniform(-s, s, (E, D)).astype(np.float32),
        "expert_b": rng.uniform(-s, s, E).astype(np.float32),
        "k": 2,
    }
    got = kernel(**inputs)
    print("kernel output:", got.shape, got.dtype, got[:4, 0])
